# revision 1
# baseline (speedup 1.0000x reference)
"""DeepSATConv GNN message-passing kernel for 8 Trainium2 NeuronCores.

Math note: the reference computes a per-channel segment-softmax over
msg = self_h[src] + neib_h[dst].  Within a dst-segment, neib_h[dst] (and
b_self, b_nb) are constant per channel, so they cancel in the softmax.
Hence alpha = segsoftmax(h[src] @ W_self.T) exactly, and
out[n] = segsum(e * h[src]) / segsum(e)  with e = exp((h @ W_self.T)[src]),
falling back to h[n] for zero-in-degree nodes.  W_nb / b_nb / b_self do
not affect the output at all.

Sharding: nodes are split across the 8 cores (2500 each); edges are
partitioned by destination node so segment reductions stay core-local;
h is replicated (the "halo gather" degenerates to replication).

Per core the kernel
  A) computes Z = h @ [W_self.T | I] = [self_h | h] for all nodes into
     core-local HBM (replicated compute; cheaper than collectives, and
     packing h alongside self_h lets one dma_gather descriptor fetch
     both operands per edge — SWDGE descriptor generation on the Q7 is
     the dominant cost of gathers),
  B) for each 128-node tile, dma_gathers Z[src] for the tile's
     (dst-sorted, padded) edge list, then for each 128-edge chunk
     builds a one-hot selector S[e, n] = (dst_local[e] == n) on the DVE
     and accumulates  [denom | numer] = S.T @ [exp(sh) | exp(sh) * hs]
     into a PSUM bank over all chunks of the tile,
  C) finalizes out = numer / max(denom, tiny), with copy_predicated
     restoring h for empty nodes, and writes the tile to HBM.
"""

import os
import numpy as np

N_NODES = 20000
N_EDGES = 320000
D = 256
CORES = 8
NPC = N_NODES // CORES          # 2500 nodes per core
NT = (NPC + 127) // 128         # 20 node tiles per core
NROWS = NT * 128                # 2560 padded rows per core
NT_ALL = 160                    # phase-A tiles (two 80-tile Z blocks)
NPAD = NT_ALL * 128             # 20480
NPB = 2                         # Z source blocks (phase A/B overlap)
NBH = NPAD // NPB               # rows per Z block
BB = 6                          # chunks per exp/mult batch

# float32r runs the selector matmul at 4x the fp32 rate but rounds
# operands to ~tf32 precision (~8e-4 output error vs ~3e-5 for fp32).
USE_F32R = os.environ.get("GNN_F32R", "0") == "1"

_cache = {}


def _build(caps):
    import concourse.bacc as bacc
    import concourse.mybir as mybir
    from concourse.tile import TileContext

    nc = bacc.Bacc("TRN2")
    f32 = mybir.dt.float32
    mm_dt = mybir.dt.float32r if USE_F32R else f32

    bf16 = mybir.dt.bfloat16
    NCH = sum(sum(r) for r in caps)     # total chunks across tiles/blocks
    NIX = 128 * NCH                     # total gathered edge slots
    hT_d = nc.dram_tensor("hT", [128, 2, 2, NPAD], bf16, kind="ExternalInput")
    WI_d = nc.dram_tensor("WI", [128, 2, 2, 2 * D], bf16, kind="ExternalInput")
    idx_d = nc.dram_tensor("idx", [128, NIX // 16], mybir.dt.int16, kind="ExternalInput")
    S_d = nc.dram_tensor("S", [128, NCH, 128], f32, kind="ExternalInput")
    hown_d = nc.dram_tensor("hown", [NROWS, D], f32, kind="ExternalInput")
    out_d = nc.dram_tensor("out", [NROWS, D], f32, kind="ExternalOutput")

    CMAX = max(a + b for a, b in caps)
    with TileContext(nc) as tc:
        with (
            tc.tile_pool(name="const", bufs=1) as constp,
            tc.tile_pool(name="pha", bufs=3) as pha,
            tc.tile_pool(name="gat", bufs=2) as gat,
            tc.tile_pool(name="wrk", bufs=6) as wrk,
            tc.tile_pool(name="fin", bufs=2) as fin,
            tc.tile_pool(name="psa", bufs=2, space="PSUM") as psa,
            tc.tile_pool(name="psb", bufs=3, space="PSUM") as psb,
            tc.tile_pool(name="dram", bufs=1, space="DRAM") as dramp,
        ):
            z_blk = []
            for s_ in range(NPB):
                zb = dramp.tile([NBH, 2 * D], f32, tag=f"zblk{s_}")
                z_blk.append(zb)

            # ---- phase A: Z = h @ [W_self.T | I] = [self_h | h], all nodes ----
            # bf16 hi/lo split: h = hi + lo, W.T columns split likewise into
            # WI_hi = [W_hi.T | I], WI_lo = [W_lo.T | 0]; three bf16 products
            # hi@WI_hi + hi@WI_lo + lo@WI_hi reproduce fp32 to ~1e-5.
            WI_sb = constp.tile([128, 2, 2, 2 * D], bf16)
            nc.sync.dma_start(WI_sb[:, :, :, :], WI_d[:, :, :, :])
            for i in range(NT_ALL):
                hT_sb = pha.tile([128, 2, 2, 128], bf16, tag="hT")
                nc.sync.dma_start(hT_sb[:, :, :, :], hT_d[:, :, :, i * 128:(i + 1) * 128])
                ps = psa.tile([128, 2 * D], f32, tag="ps")
                nmm = 0
                for hw, ww in ((0, 0), (0, 1), (1, 0)):
                    for kb in range(2):
                        nc.tensor.matmul(
                            ps[:, :], hT_sb[:, hw, kb, :], WI_sb[:, ww, kb, :],
                            start=(nmm == 0), stop=(nmm == 5),
                        )
                        nmm += 1
                z_sb = pha.tile([128, 2 * D], f32, tag="zs")
                nc.scalar.copy(z_sb[:, :], ps[:, :])
                blk, row = divmod(i * 128, NBH)
                nc.sync.dma_start(z_blk[blk][row:row + 128, :], z_sb[:, :])

            # ---- constants ----
            idx_sb = constp.tile([128, NIX // 16], mybir.dt.int16)
            nc.sync.dma_start(idx_sb[:, :], idx_d[:, :])

            # ---- phase B: per node-tile segment softmax ----
            chunk_off = 0   # global chunk counter (indexes idx/S/dstl layout)
            for t in range(NT):
                zx_t = gat.tile([128, CMAX, 2 * D], f32, tag="zx")
                C_t = caps[t][0] + caps[t][1]
                zoff = 0
                for s_ in range(NPB):
                    Cs = caps[t][s_]
                    if Cs == 0:
                        continue
                    CAPs = 128 * Cs
                    io = (chunk_off + zoff) * 8
                    nc.gpsimd.dma_gather(
                        zx_t[:, zoff:zoff + Cs, :], z_blk[s_][:, :],
                        idx_sb[:, io:io + 8 * Cs], CAPs, CAPs, 2 * D,
                        single_packet=False,
                    )
                    zoff += Cs
                acc = psb.tile([128, 2 * D], f32, tag="acc")
                for g in range((C_t + BB - 1) // BB):
                    b = min(BB, C_t - g * BB)
                    eX = wrk.tile([128, BB, 2 * D], mm_dt, tag="eX")
                    Sg = wrk.tile([128, BB, 128], f32, tag="Sg")
                    so = chunk_off + g * BB
                    nc.sync.dma_start(Sg[:, 0:b, :], S_d[:, so:so + b, :])
                    nc.scalar.activation(
                        eX[:, 0:b, 0:D], zx_t[:, g * BB:g * BB + b, 0:D],
                        mybir.ActivationFunctionType.Exp,
                    )
                    nc.vector.tensor_tensor(
                        eX[:, 0:b, D:2 * D], eX[:, 0:b, 0:D],
                        zx_t[:, g * BB:g * BB + b, D:2 * D],
                        mybir.AluOpType.mult,
                    )
                    for j in range(b):
                        k = g * BB + j
                        nc.tensor.matmul(
                            acc[:, :], Sg[:, j, :], eX[:, j, :],
                            start=(k == 0), stop=(k == C_t - 1),
                        )
                chunk_off += C_t

                # ---- finalize tile ----
                accs = fin.tile([128, 2 * D], f32, tag="accs")
                nc.scalar.copy(accs[:, :], acc[:, :])
                dmax = fin.tile([128, D], f32, tag="dmax")
                nc.vector.tensor_scalar(
                    dmax[:, :], accs[:, 0:D], 1e-37, None, mybir.AluOpType.max
                )
                rec = fin.tile([128, D], f32, tag="rec")
                nc.vector.reciprocal(rec[:, :], dmax[:, :])
                res = fin.tile([128, D], f32, tag="res")
                nc.vector.tensor_tensor(
                    res[:, :], accs[:, D:2 * D], rec[:, :], mybir.AluOpType.mult
                )
                mask = fin.tile([128, D], mybir.dt.uint8, tag="mask")
                nc.vector.tensor_scalar(
                    mask[:, :], accs[:, 0:D], 0.0, None, mybir.AluOpType.is_equal
                )
                hown_sb = fin.tile([128, D], f32, tag="hown")
                nc.sync.dma_start(hown_sb[:, :], hown_d[t * 128:(t + 1) * 128, :])
                nc.vector.copy_predicated(res[:, :], mask[:, :], hown_sb[:, :])
                nc.sync.dma_start(out_d[t * 128:(t + 1) * 128, :], res[:, :])
    nc.compile()
    return nc


def _wrap_idx(ix):
    # dma_gather index layout: logical index i lands at output
    # [partition i%128, slot i//128]; the SBUF index tile stores it at
    # [i%16, 8*(i//128) + (i%128)//16], replicated over the 8 Q7 cores.
    w = ix.astype(np.int16).reshape(-1, 8, 16).transpose(2, 0, 1).reshape(16, -1)
    return np.tile(w, (8, 1))


def kernel(h, W_nb, b_nb, W_self, b_self, src, dst):
    from concourse.bass_utils import run_bass_kernel_spmd

    h = np.ascontiguousarray(np.asarray(h, dtype=np.float32))
    W = np.asarray(W_self, dtype=np.float32)
    src = np.asarray(src, dtype=np.int64)
    dst = np.asarray(dst, dtype=np.int64)

    order = np.argsort(dst, kind="stable")
    src_s = src[order]
    dst_s = dst[order]

    # per-(core, tile) edge ranges; tiles are 128 consecutive owned nodes
    tile_base = []
    for c in range(CORES):
        for t in range(NT):
            tile_base.append(c * NPC + t * 128)
    bounds_lo = np.searchsorted(dst_s, np.array(tile_base), side="left")
    hi_nodes = [min(b + 128, (b // NPC + 1) * NPC) for b in tile_base]
    bounds_hi = np.searchsorted(dst_s, np.array(hi_nodes), side="left")

    # split each tile's edges by src block; caps shared across cores (SPMD)
    per_ct = {}
    cnt = np.zeros((CORES, NT, NPB), dtype=np.int64)
    for c in range(CORES):
        for t in range(NT):
            i = c * NT + t
            lo, hi = int(bounds_lo[i]), int(bounds_hi[i])
            blk = src_s[lo:hi] // NBH
            for s_ in range(NPB):
                sel = np.nonzero(blk == s_)[0]
                per_ct[(c, t, s_)] = (src_s[lo:hi][sel], dst_s[lo:hi][sel] - tile_base[i])
                cnt[c, t, s_] = len(sel)
    caps = [
        [int((cnt[:, t, s_].max() + 127) // 128) for s_ in range(NPB)]
        for t in range(NT)
    ]
    assert max(a + b for a, b in caps) <= 36, f"edge distribution too skewed: {caps}"
    NCH = sum(sum(r) for r in caps)

    # host-side layout prep: bf16 hi/lo split of h and W for phase A
    import ml_dtypes
    bf = ml_dtypes.bfloat16
    h_hi = h.astype(bf)
    h_lo = (h - h_hi.astype(np.float32)).astype(bf)
    W_hi = W.astype(bf)
    W_lo = (W - W_hi.astype(np.float32)).astype(bf)

    hT = np.zeros((2, D, NPAD), dtype=bf)
    hT[0, :, :N_NODES] = h_hi.T
    hT[1, :, :N_NODES] = h_lo.T
    hT = np.ascontiguousarray(
        hT.reshape(2, 2, 128, NPAD).transpose(2, 0, 1, 3)
    )
    WI = np.zeros((2, D, 2 * D), dtype=bf)
    WI[0, :, :D] = W_hi.T
    WI[1, :, :D] = W_lo.T
    WI[0, np.arange(D), D + np.arange(D)] = bf(1.0)
    WI = np.ascontiguousarray(
        WI.reshape(2, 2, 128, 2 * D).transpose(2, 0, 1, 3)
    )

    in_maps = []
    for c in range(CORES):
        idx_parts = []
        S_all = np.zeros((128, NCH, 128), dtype=np.float32)
        coff = 0
        for t in range(NT):
            for s_ in range(NPB):
                Cs = caps[t][s_]
                if Cs == 0:
                    continue
                CAPs = 128 * Cs
                ss, dl_real = per_ct[(c, t, s_)]
                n = len(ss)
                spad = np.zeros(CAPs, dtype=np.int64)
                spad[:n] = ss - s_ * NBH      # block-local row index
                dl = np.full(CAPs, -1, dtype=np.int64)
                dl[:n] = dl_real
                idx_parts.append(_wrap_idx(spad))
                ei = np.nonzero(dl >= 0)[0]
                S_all[ei % 128, coff + ei // 128, dl[ei]] = 1.0
                coff += Cs
        hown = np.zeros((NROWS, D), dtype=np.float32)
        hown[:NPC] = h[c * NPC:(c + 1) * NPC]
        in_maps.append({
            "hT": hT,
            "WI": WI,
            "idx": np.ascontiguousarray(np.concatenate(idx_parts, axis=1)),
            "S": S_all,
            "hown": hown,
        })

    key = tuple(tuple(r) for r in caps)
    if key not in _cache:
        _cache[key] = _build(caps)
    nc = _cache[key]

    res = run_bass_kernel_spmd(nc, in_maps, core_ids=list(range(CORES)))
    out = np.concatenate(
        [res.results[c]["out"][:NPC] for c in range(CORES)], axis=0
    )
    return out.astype(np.float32)



# revision 2
# speedup vs baseline: 1.4981x; 1.4981x over previous
"""DeepSATConv GNN message-passing kernel for 8 Trainium2 NeuronCores.

Math note: the reference computes a per-channel segment-softmax over
msg = self_h[src] + neib_h[dst].  Within a dst-segment, neib_h[dst] (and
b_self, b_nb) are constant per channel, so they cancel in the softmax.
Hence alpha = segsoftmax(h @ W_self.T) exactly, and
out[n] = segsum(e * h[src]) / segsum(e)  with e = exp((h @ W_self.T)[src]),
falling back to h[n] for zero-in-degree nodes.  W_nb / b_nb / b_self do
not affect the output at all.

Sharding: nodes are split across the 8 cores (2500 each); edges are
partitioned by destination node so segment reductions stay core-local;
h is replicated (the "halo gather" degenerates to replication).

v2 design (vs the fp32 baseline): everything that feeds the tensor
engine is bf16 (4x the fp32 matmul rate), and the gathered row packs
[e | h] in bf16 (1024B descriptors, half the baseline's bytes).

  A) Z = [exp(h_hi @ W_hi.T) | h] as a [NPAD, 512] bf16 table.  The
     h-columns are pre-filled by the host (Z is an ExternalInput); the
     device computes only the e-columns: per quad of 128-node tiles,
     2 bf16 matmuls per tile (K=256 split in two) into PSUM, one Exp
     activation (f32->bf16), one strided DMA into Z[:, 0:256].
  B) per 128-node tile, one dma_gather fetches Z[src] (1024B rows) for
     the tile's dst-sorted padded edge list; the one-hot selector
     S[e, n] = (dst_local[e] == n) is built on the DVE from an iota
     constant and a per-chunk dst column (no S matrix from HBM); the
     DVE also forms eh = e*h; the PE accumulates
     [denom | numer] = S.T @ [e | eh] into one PSUM bank per tile.
  C) finalize out = numer / max(denom, tiny) with copy_predicated
     restoring h for zero-degree nodes; bf16 output.

Numerics (validated against the jax reference in fp64-free numpy):
bf16 tables + bf16 selector matmul + W_hi-only phase A give ~3.5e-3
relative error vs the 2e-2 budget.
"""

import os
import numpy as np

N_NODES = 20000
N_EDGES = 320000
D = 256
CORES = 8
NPC = N_NODES // CORES          # 2500 nodes per core
NT = (NPC + 127) // 128         # 20 node tiles per core
NROWS = NT * 128                # 2560 padded rows per core
NT_ALL = 160                    # phase-A 128-node tiles over all nodes
NPAD = NT_ALL * 128             # 20480
QT = 4                          # phase-A tiles per iteration (one PSUM pair)
BB = 6                          # chunks per DVE mult batch

SINGLE_PACKET = os.environ.get("GNN_SP", "0") == "1"

_cache = {}


def _build(caps):
    import concourse.bacc as bacc
    import concourse.mybir as mybir
    from concourse.tile import TileContext

    nc = bacc.Bacc("TRN2")
    f32 = mybir.dt.float32
    bf16 = mybir.dt.bfloat16

    NCH = sum(caps)                     # total chunks across tiles
    NIX = 128 * NCH                     # total gathered edge slots
    CMAX = max(caps)

    hT_d = nc.dram_tensor("hT", [128, 2, NPAD], bf16, kind="ExternalInput")
    WT_d = nc.dram_tensor("WT", [128, 2, D], bf16, kind="ExternalInput")
    Z_d = nc.dram_tensor("Z", [NPAD, 2 * D], bf16, kind="ExternalInput")
    idx_d = nc.dram_tensor("idx", [128, NIX // 16], mybir.dt.int16, kind="ExternalInput")
    dstl_d = nc.dram_tensor("dstl", [128, NCH], f32, kind="ExternalInput")
    hown_d = nc.dram_tensor("hown", [NROWS, D], bf16, kind="ExternalInput")
    out_d = nc.dram_tensor("out", [NROWS, D], bf16, kind="ExternalOutput")

    with TileContext(nc) as tc:
        with (
            tc.tile_pool(name="const", bufs=1) as constp,
            tc.tile_pool(name="pha", bufs=3) as pha,
            tc.tile_pool(name="gat", bufs=2) as gat,
            tc.tile_pool(name="wrk", bufs=2) as wrk,
            tc.tile_pool(name="fin", bufs=2) as fin,
            tc.tile_pool(name="psa", bufs=2, space="PSUM") as psa,
            tc.tile_pool(name="psb", bufs=3, space="PSUM") as psb,
        ):
            # ---- constants ----
            WT_sb = constp.tile([128, 2, D], bf16)
            nc.sync.dma_start(WT_sb[:, :, :], WT_d[:, :, :])
            idx_sb = constp.tile([128, NIX // 16], mybir.dt.int16)
            nc.sync.dma_start(idx_sb[:, :], idx_d[:, :])
            dstl_sb = constp.tile([128, NCH], f32)
            nc.sync.dma_start(dstl_sb[:, :], dstl_d[:, :])
            iota_sb = constp.tile([128, 128], f32)
            nc.gpsimd.iota(
                iota_sb[:, :], [[1, 128]], base=0, channel_multiplier=0,
                allow_small_or_imprecise_dtypes=True,
            )

            # ---- phase A: e-columns of Z, QT node-tiles per iteration ----
            for i in range(NT_ALL // QT):
                hT_sb = pha.tile([128, 2, QT * 128], bf16, tag="hT")
                nc.sync.dma_start(
                    hT_sb[:, :, :], hT_d[:, :, i * QT * 128:(i + 1) * QT * 128]
                )
                ps = psa.tile([128, QT, D], f32, tag="ps")
                for u in range(QT):
                    for kb in range(2):
                        nc.tensor.matmul(
                            ps[:, u, :],
                            hT_sb[:, kb, u * 128:(u + 1) * 128],
                            WT_sb[:, kb, :],
                            start=(kb == 0), stop=(kb == 1),
                        )
                e_sb = pha.tile([128, QT, D], bf16, tag="es")
                nc.scalar.activation(
                    e_sb[:, :, :], ps[:, :, :], mybir.ActivationFunctionType.Exp
                )
                zrows = Z_d[i * QT * 128:(i + 1) * QT * 128, 0:D]
                nc.sync.dma_start(
                    zrows.rearrange("(u p) c -> p u c", p=128), e_sb[:, :, :]
                )

            # ---- phase B: per node-tile gather + segment softmax ----
            coff = 0   # global chunk counter (indexes idx/dstl layout)
            for t in range(NT):
                C = caps[t]
                zx = gat.tile([128, CMAX, 2 * D], bf16, tag="zx")
                nc.gpsimd.dma_gather(
                    zx[:, 0:C, :], Z_d[:, :],
                    idx_sb[:, coff * 8:(coff + C) * 8], 128 * C, 128 * C, 2 * D,
                    single_packet=SINGLE_PACKET,
                )
                S_t = wrk.tile([128, CMAX, 128], bf16, tag="S")
                for j in range(C):
                    nc.vector.tensor_scalar(
                        S_t[:, j, :], iota_sb[:, :],
                        dstl_sb[:, coff + j:coff + j + 1], None,
                        mybir.AluOpType.is_equal,
                    )
                ehx = wrk.tile([128, CMAX, D], bf16, tag="ehx")
                for g in range((C + BB - 1) // BB):
                    b = min(BB, C - g * BB)
                    nc.vector.tensor_tensor(
                        ehx[:, g * BB:g * BB + b, :],
                        zx[:, g * BB:g * BB + b, 0:D],
                        zx[:, g * BB:g * BB + b, D:2 * D],
                        mybir.AluOpType.mult,
                    )
                acc = psb.tile([128, 2 * D], f32, tag="acc")
                for j in range(C):
                    nc.tensor.matmul(
                        acc[:, 0:D], S_t[:, j, :], zx[:, j, 0:D],
                        start=(j == 0), stop=(j == C - 1),
                    )
                for j in range(C):
                    nc.tensor.matmul(
                        acc[:, D:2 * D], S_t[:, j, :], ehx[:, j, :],
                        start=(j == 0), stop=(j == C - 1),
                    )
                coff += C

                # ---- finalize tile ----
                accs = fin.tile([128, 2 * D], f32, tag="accs")
                nc.scalar.copy(accs[:, :], acc[:, :])
                dmax = fin.tile([128, D], f32, tag="dmax")
                nc.vector.tensor_scalar(
                    dmax[:, :], accs[:, 0:D], 1e-37, None, mybir.AluOpType.max
                )
                rec = fin.tile([128, D], f32, tag="rec")
                nc.vector.reciprocal(rec[:, :], dmax[:, :])
                res = fin.tile([128, D], bf16, tag="res")
                nc.vector.tensor_tensor(
                    res[:, :], accs[:, D:2 * D], rec[:, :], mybir.AluOpType.mult
                )
                mask = fin.tile([128, D], mybir.dt.uint8, tag="mask")
                nc.vector.tensor_scalar(
                    mask[:, :], accs[:, 0:D], 0.0, None, mybir.AluOpType.is_equal
                )
                hown_sb = fin.tile([128, D], bf16, tag="hown")
                nc.sync.dma_start(hown_sb[:, :], hown_d[t * 128:(t + 1) * 128, :])
                nc.vector.copy_predicated(res[:, :], mask[:, :], hown_sb[:, :])
                nc.sync.dma_start(out_d[t * 128:(t + 1) * 128, :], res[:, :])
    nc.compile()
    return nc


def _wrap_idx(ix):
    # dma_gather index layout: logical index i lands at output
    # [partition i%128, slot i//128]; the SBUF index tile stores it at
    # [i%16, 8*(i//128) + (i%128)//16], replicated over the 8 Q7 cores.
    w = ix.astype(np.int16).reshape(-1, 8, 16).transpose(2, 0, 1).reshape(16, -1)
    return np.tile(w, (8, 1))


def kernel(h, W_nb, b_nb, W_self, b_self, src, dst):
    from concourse.bass_utils import run_bass_kernel_spmd
    import ml_dtypes

    bf = ml_dtypes.bfloat16
    h = np.ascontiguousarray(np.asarray(h, dtype=np.float32))
    W = np.asarray(W_self, dtype=np.float32)
    src = np.asarray(src, dtype=np.int64)
    dst = np.asarray(dst, dtype=np.int64)

    order = np.argsort(dst, kind="stable")
    src_s = src[order]
    dst_s = dst[order]

    # per-(core, tile) edge ranges; tiles are 128 consecutive owned nodes
    tile_base = []
    for c in range(CORES):
        for t in range(NT):
            tile_base.append(c * NPC + t * 128)
    bounds_lo = np.searchsorted(dst_s, np.array(tile_base), side="left")
    hi_nodes = [min(b + 128, (b // NPC + 1) * NPC) for b in tile_base]
    bounds_hi = np.searchsorted(dst_s, np.array(hi_nodes), side="left")

    cnt = np.zeros((CORES, NT), dtype=np.int64)
    for c in range(CORES):
        for t in range(NT):
            cnt[c, t] = bounds_hi[c * NT + t] - bounds_lo[c * NT + t]
    caps = [int((cnt[:, t].max() + 127) // 128) for t in range(NT)]
    assert max(caps) <= 22, f"edge distribution too skewed: {caps}"
    NCH = sum(caps)

    # host-side layout prep
    h_bf = h.astype(bf)
    hT = np.zeros((128, 2, NPAD), dtype=bf)
    hT[:, :, :N_NODES] = np.ascontiguousarray(
        h_bf.T.reshape(2, 128, N_NODES).transpose(1, 0, 2)
    )
    WT = np.ascontiguousarray(
        W.astype(bf).T.reshape(2, 128, D).transpose(1, 0, 2)
    )
    Z = np.zeros((NPAD, 2 * D), dtype=bf)
    Z[:N_NODES, D:2 * D] = h_bf

    in_maps = []
    for c in range(CORES):
        idx_parts = []
        dstl = np.full((128, NCH), -1.0, dtype=np.float32)
        coff = 0
        for t in range(NT):
            Ct = caps[t]
            CAPs = 128 * Ct
            i = c * NT + t
            lo, hi = int(bounds_lo[i]), int(bounds_hi[i])
            n = hi - lo
            spad = np.zeros(CAPs, dtype=np.int64)
            spad[:n] = src_s[lo:hi]
            idx_parts.append(_wrap_idx(spad))
            ei = np.arange(n)
            dstl[ei % 128, coff + ei // 128] = (dst_s[lo:hi] - tile_base[i]).astype(
                np.float32
            )
            coff += Ct
        hown = np.zeros((NROWS, D), dtype=bf)
        hown[:NPC] = h_bf[c * NPC:(c + 1) * NPC]
        in_maps.append({
            "hT": hT,
            "WT": WT,
            "Z": Z,
            "idx": np.ascontiguousarray(np.concatenate(idx_parts, axis=1)),
            "dstl": dstl,
            "hown": hown,
        })

    key = tuple(caps)
    if key not in _cache:
        _cache[key] = _build(caps)
    nc = _cache[key]

    res = run_bass_kernel_spmd(nc, in_maps, core_ids=list(range(CORES)))
    out = np.concatenate(
        [res.results[c]["out"][:NPC] for c in range(CORES)], axis=0
    )
    return out.astype(np.float32)
